# revision 3
# baseline (speedup 1.0000x reference)
"""GATv2 3-layer GNN on 8 Trainium2 NeuronCores (Bass/Tile) — v2.1.

Structure (per core, nodes dst-sharded, edges by dst, dst-sorted):
  - phase A (per 128-node block): project [xl|xr] = x @ [Wl|Wr] + b (PE),
    xl staged to cc_in DRAM, xr resident in SBUF.
  - one AllGather per layer: cc_in [NP,D] -> tab [8*NP,D] (core-major, so
    rows < 4*NP are cores 0-3 => int16-indexable table halves).
  - edge phase (per block, TB_b exact 128-edge columns, slots grouped by
    src-core-half): 2x dma_gather pulls xl[src] rows into SBUF (the only
    per-edge descriptor work, ~7.5ns/row on the Q7 — the kernel's
    bottleneck); PE adds xr[dst] via fp8 one-hot mT matmul + fp8 identity
    matmul of the gathered xl into one PSUM accumulation; ACT evacuates
    v = xl+xr to SBUF bf16.
  - logits: lrelu via one DVE scalar_tensor_tensor, att dot via DVE
    mult+reduce, exp on ACT; payload weighting on DVE; aggregation via
    fp8 one-hot mE matmuls (PE) accumulating [sum_e exp | sum_e exp*v].
  - node side: out = num/den - xr (v-trick, sum alpha == 1), bias, ELU.
  - layer 2 (D=64) runs 128-wide with zero-padded table/att columns and
    feeds a per-core global-mean-pool partial; host sums and divides.
"""

import sys

if "/opt/trn_rl_repo" not in sys.path:
    sys.path.insert(0, "/opt/trn_rl_repo")

import numpy as np
import ml_dtypes

BF16 = ml_dtypes.bfloat16
FP8 = ml_dtypes.float8_e4m3

NEG_SLOPE = 0.2
N_NODES = 50000
N_EDGES = 800000
N_GRAPHS = 64
IN_CH = 128
HIDDEN = 128
HEADS = 4
OUT_CH = 64
NCORES = 8

NPC = N_NODES // NCORES          # 6250
NP = ((NPC + 127) // 128) * 128  # 6272
NBLK = NP // 128                 # 49
DW = 128                         # table/edge channel width (l2 zero-padded)
LAYERS = [
    (IN_CH, HIDDEN, HEADS, HIDDEN // HEADS, True),
    (HIDDEN, HIDDEN, HEADS, HIDDEN // HEADS, True),
    (HIDDEN, OUT_CH, 1, OUT_CH, False),
]


# ---------------------------------------------------------------- host prep
def prep(x, edge_index, batch):
    src = np.asarray(edge_index[0], dtype=np.int64)
    dst = np.asarray(edge_index[1], dtype=np.int64)
    batch = np.asarray(batch, dtype=np.int64)
    x = np.asarray(x, dtype=np.float32)

    core_of = dst // NPC
    dloc = (dst % NPC).astype(np.int64)
    bloc = dloc // 128
    src_core = src // NPC
    src_r = src % NPC
    trow = src_core * NP + src_r          # core-major table row
    lowhalf = (src_core < NCORES // 2)
    trow_h = np.where(lowhalf, trow, trow - (NCORES // 2) * NP)  # < 25088

    # group sizes -> shared per-block column counts (nA | nB per block)
    nAB = np.zeros((NCORES, NBLK, 2), np.int64)
    for c in range(NCORES):
        sel = core_of == c
        for half in (0, 1):
            s2 = sel & (lowhalf == (half == 0))
            nAB[c, :, half] = np.bincount(bloc[s2], minlength=NBLK)
    nA = np.maximum(1, (nAB[:, :, 0].max(axis=0) + 127) // 128)
    nB = np.maximum(1, (nAB[:, :, 1].max(axis=0) + 127) // 128)
    kmax = np.maximum(1, nAB.max(axis=0))  # [NBLK, 2] shared valid-idx count
    tbs = tuple(int(a + b) for a, b in zip(nA, nB))
    splits = tuple(int(a) for a in nA)
    offs = np.concatenate([[0], np.cumsum(tbs)]).astype(int)
    tot_tb = int(offs[-1])

    maps = []
    for c in range(NCORES):
        sel = core_of == c
        eh = lowhalf[sel].astype(np.int64) ^ 1   # 0=low, 1=high
        es = trow_h[sel]
        ed = dloc[sel]
        eb = bloc[sel]
        # sort by (block, half, dst) -> slots grouped per block: [low | high]
        order = np.lexsort((ed, eh, eb))
        es, ed, eb, eh = es[order], ed[order], eb[order], eh[order]

        idx16 = np.zeros((16, tot_tb * 8), np.int16)  # i at (i%16, i//16)
        mE = np.zeros((128, tot_tb * 128), FP8)
        mT = np.zeros((128, tot_tb * 128), FP8)
        for b in range(NBLK):
            o = int(offs[b])
            for half, cols, coff in ((0, int(nA[b]), 0), (1, int(nB[b]), int(nA[b]))):
                s2 = (eb == b) & (eh == half)
                k = int(s2.sum())
                rows = es[s2]
                dsts = ed[s2] - 128 * b
                n = cols * 128
                km = int(kmax[b, half])
                ids = np.full(n, -1, np.int16)  # -1 tail: Q7 skips emission
                ids[:km] = 0  # valid filler rows up to the shared count
                ids[:k] = rows
                colbase = (o + coff) * 8
                idx16[:, colbase : colbase + cols * 8] = ids.reshape(cols * 8, 16).T
                ks = np.arange(k)
                p = ks % 128
                t = ks // 128 + coff
                mE_blk = np.zeros((128, cols, 128), np.float32)
                mE_blk[p, t - coff, dsts] = 1.0
                mE[:, (o + coff) * 128 : (o + coff + cols) * 128] = (
                    mE_blk.reshape(128, cols * 128).astype(FP8))
                mT_blk = np.transpose(mE_blk, (2, 1, 0))  # [node, t, slot]
                mT[:, (o + coff) * 128 : (o + coff + cols) * 128] = (
                    mT_blk.reshape(128, cols * 128).astype(FP8))

        idx16_full = np.broadcast_to(
            idx16.reshape(1, 16, tot_tb * 8), (8, 16, tot_tb * 8)
        ).reshape(128, tot_tb * 8).copy()

        pm = np.zeros((NP, N_GRAPHS), np.float32)
        pm[np.arange(NPC), batch[c * NPC : (c + 1) * NPC]] = 1.0

        xs = np.zeros((IN_CH, NP), np.float32)
        xs[:, :NPC] = x[c * NPC : (c + 1) * NPC].T

        maps.append(
            dict(
                x0T=xs.astype(BF16),
                idx16=idx16_full,
                mE=mE,
                mT=mT,
                pool_mask=pm.astype(BF16),
            )
        )

    counts = np.bincount(batch, minlength=N_GRAPHS).astype(np.float32)
    kreals = tuple((int(kmax[b, 0]), int(kmax[b, 1])) for b in range(NBLK))
    return maps, counts, tbs, splits, kreals


def prep_weights(inp):
    w = {}
    for l, (Din, D, H, C, _) in enumerate(LAYERS):
        Wl = np.asarray(inp[f"Wl{l}"], np.float32)
        bl = np.asarray(inp[f"bl{l}"], np.float32)
        Wr = np.asarray(inp[f"Wr{l}"], np.float32)
        br = np.asarray(inp[f"br{l}"], np.float32)
        att = np.asarray(inp[f"att{l}"], np.float32)
        bo = np.asarray(inp[f"bias{l}"], np.float32)
        wc = np.zeros((Din, 2 * DW), np.float32)
        wc[:, :D] = Wl
        wc[:, DW : DW + D] = Wr
        w[f"wcat{l}"] = wc.astype(BF16)
        be = np.zeros(2 * DW, np.float32)
        be[:D] = bl
        be[DW : DW + D] = br
        w[f"bias_in{l}"] = np.broadcast_to(
            (be / 128.0)[None, :], (128, 2 * DW)).astype(BF16).copy()
        ae = np.zeros(DW, np.float32)
        ae[: H * C] = att.reshape(H * C)
        w[f"att{l}"] = np.broadcast_to(ae[None, :], (128, DW)).astype(BF16).copy()
        w[f"bias_out{l}"] = np.broadcast_to(bo[None, :], (128, D)).copy()
    w["ident8"] = np.eye(128, dtype=np.float32).astype(FP8)
    w["ones8"] = np.ones((128, 128), np.float32).astype(FP8)
    w["identf"] = np.eye(128, dtype=np.float32)
    return w


# ---------------------------------------------------------------- device build
def build(tbs, splits, kreals):
    from concourse import bass, bacc, mybir
    import concourse.tile as tile
    from concourse.tile import add_dep_helper

    F32 = mybir.dt.float32
    BF = mybir.dt.bfloat16
    I16 = mybir.dt.int16
    F8 = mybir.dt.float8e4
    A = mybir.AluOpType
    ACTF = mybir.ActivationFunctionType

    TOT_TB = sum(tbs)
    TBMAX = max(tbs)
    offs = np.concatenate([[0], np.cumsum(tbs)]).astype(int)
    G = N_GRAPHS
    HALF_ROWS = (NCORES // 2) * NP  # 25088

    nc = bacc.Bacc(
        "TRN2",
        target_bir_lowering=False,
        debug=False,
        enable_asserts=False,
        num_devices=NCORES,
    )

    def ein(name, shape, dt):
        return nc.dram_tensor(name, shape, dt, kind="ExternalInput").ap()

    x0T = ein("x0T", [IN_CH, NP], BF)
    idx16_d = ein("idx16", [128, TOT_TB * 8], I16)
    mE_d = ein("mE", [128, TOT_TB * 128], F8)
    mT_d = ein("mT", [128, TOT_TB * 128], F8)
    pool_mask = ein("pool_mask", [NP, G], BF)
    ident8_d = ein("ident8", [128, 128], F8)
    ones8_d = ein("ones8", [128, 128], F8)
    identf_d = ein("identf", [128, 128], F32)
    wcat_d, biasin_d, att_d, biasout_d = [], [], [], []
    for l, (Din, D, H, C, _) in enumerate(LAYERS):
        wcat_d.append(ein(f"wcat{l}", [Din, 2 * DW], BF))
        biasin_d.append(ein(f"bias_in{l}", [128, 2 * DW], BF))
        att_d.append(ein(f"att{l}", [128, DW], BF))
        biasout_d.append(ein(f"bias_out{l}", [128, D], F32))

    pool_out = nc.dram_tensor("pool_out", [G, OUT_CH], F32, kind="ExternalOutput").ap()

    cc_in, tabs = [], []
    for l in range(3):
        cc_in.append(nc.dram_tensor(f"cc_in{l}", [NP, DW], BF, kind="Internal").ap())
        tabs.append(
            nc.dram_tensor(f"tab{l}", [NCORES * NP, DW], BF, kind="Internal",
                           addr_space="Shared").ap())

    from contextlib import ExitStack

    with tile.TileContext(nc) as tc, ExitStack() as pools:
        const = pools.enter_context(tc.tile_pool(name="const", bufs=1))
        work = pools.enter_context(tc.tile_pool(name="work", bufs=2))
        nodep = pools.enter_context(tc.tile_pool(name="nodep", bufs=2))
        psum_t = pools.enter_context(tc.tile_pool(name="psum_t", bufs=1, space="PSUM"))
        psum_a = pools.enter_context(tc.tile_pool(name="psum_a", bufs=1, space="PSUM"))
        psum_v = pools.enter_context(tc.tile_pool(name="psum_v", bufs=1, space="PSUM"))
        psum_g = pools.enter_context(tc.tile_pool(name="psum_g", bufs=1, space="PSUM"))
        psum_p = pools.enter_context(tc.tile_pool(name="psum_p", bufs=1, space="PSUM"))

        h_sb = nc.alloc_sbuf_tensor("h_sb", [128, NBLK, HIDDEN], F32).ap()
        xr_sb = nc.alloc_sbuf_tensor("xr_sb", [128, NBLK, DW], BF).ap()
        NAMAX = max(splits)
        NBMAX = max(t - s for t, s in zip(tbs, splits))
        vA = nc.alloc_sbuf_tensor("vA", [128, 3, NAMAX, DW], BF).ap()
        vB = nc.alloc_sbuf_tensor("vB", [128, 3, NBMAX, DW], BF).ap()
        nc.vector.memset(vA[:], 0.0)
        nc.vector.memset(vB[:], 0.0)

        def const_tile(shape, dt, src_ap, tag):
            t = const.tile(shape, dt, tag=tag)
            nc.sync.dma_start(out=t[:], in_=src_ap)
            return t

        ident8 = const_tile([128, 128], F8, ident8_d, "id8")
        ones8 = const_tile([128, 128], F8, ones8_d, "ones8")
        identf = const_tile([128, 128], F32, identf_d, "idf")
        idx16 = const_tile([128, TOT_TB * 8], I16, idx16_d, "idx16")
        wcat_s, biasin_s, attrep_s, biasout_s = [], [], [], []
        for l, (Din, D, H, C, _) in enumerate(LAYERS):
            wcat_s.append(const_tile([Din, 2 * DW], BF, wcat_d[l][:], f"wc{l}"))
            biasin_s.append(const_tile([128, 2 * DW], BF, biasin_d[l][:], f"bi{l}"))
            biasout_s.append(const_tile([128, D], F32, biasout_d[l][:], f"bo{l}"))
            at1 = const.tile([128, DW], BF, tag=f"at{l}")
            nc.sync.dma_start(out=at1[:], in_=att_d[l][:])
            ar = const.tile([128, TBMAX, DW], BF, tag=f"ar{l}")
            nc.vector.tensor_scalar(
                out=ar[:],
                in0=at1[:].rearrange("p (o d) -> p o d", o=1).to_broadcast(
                    [128, TBMAX, DW]),
                scalar1=0.5 + 0.5 * NEG_SLOPE, scalar2=None, op0=A.mult)
            attrep_s.append(ar)

        cc_ops = {0: [], 1: [], 2: []}

        def phase_a(l, b, cc_writes):
            Din, D, H, C, _ = LAYERS[l]
            if l == 0:
                xT = nodep.tile([128, 128], BF, tag="pa_xT")
                nc.sync.dma_start(out=xT[:], in_=x0T[:, b * 128 : (b + 1) * 128])
            else:
                tp = psum_t.tile([128, 128], F32, tag="pa_tr")
                nc.tensor.transpose(out=tp[:], in_=h_sb[:, b, :Din], identity=identf[:])
                xT = nodep.tile([128, 128], BF, tag="pa_xT")
                nc.scalar.copy(out=xT[:], in_=tp[:])
            pp = psum_a.tile([128, 2 * DW], F32, tag="pa_mm")
            nc.tensor.matmul(out=pp[:], lhsT=xT[:], rhs=wcat_s[l][:],
                             start=True, stop=False)
            nc.tensor.matmul(out=pp[:], lhsT=ones8[:], rhs=biasin_s[l][:],
                             start=False, stop=True)
            xl_t = nodep.tile([128, DW], BF, tag="pa_xl")
            nc.scalar.copy(out=xl_t[:], in_=pp[:, :DW])
            nc.scalar.copy(out=xr_sb[:, b, :], in_=pp[:, DW:])
            w1 = nc.sync.dma_start(out=cc_in[l][b * 128 : (b + 1) * 128, :], in_=xl_t[:])
            cc_writes.append((b, w1))

        def fire_cc(l, cc_writes):
            cc = nc.gpsimd.collective_compute(
                "AllGather",
                A.bypass,
                replica_groups=[list(range(NCORES))],
                ins=[cc_in[l][:]],
                outs=[tabs[l][:]],
            )
            for bb, w1 in cc_writes:
                add_dep_helper(cc.ins, w1.ins, sync=True,
                               reason="cc after shard write")
            cc_ops[l].append(cc)

        # ---------------- layer 0 phase A
        ccw = []
        for b in range(NBLK):
            phase_a(0, b, ccw)
        fire_cc(0, ccw)

        # ---------------- per-layer edge loop
        for l, (Din, D, H, C, use_elu) in enumerate(LAYERS):
            W = H + DW
            ccw = []

            def emit_gather(b, l=l):
                s = b % 3
                tb = tbs[b]
                nA = splits[b]
                nB = tb - nA
                o = int(offs[b])
                for half, cols, coff, base, vbuf in (
                    (0, nA, 0, 0, vA), (1, nB, nA, HALF_ROWS, vB)
                ):
                    n = cols * 128
                    g = nc.gpsimd.dma_gather(
                        vbuf[:, s, :cols, :],
                        tabs[l][base : base + HALF_ROWS, :],
                        idx16[:, (o + coff) * 8 : (o + coff + cols) * 8],
                        n, int(kreals[b][half]), DW,
                        single_packet=False,
                    )
                    for cc in cc_ops[l]:
                        add_dep_helper(g.ins, cc.ins, sync=True,
                                       reason="gather after cc")

            if l == 2:
                pool_ps = psum_p.tile([G, OUT_CH], F32, tag="pool")

            emit_gather(0)
            for b in range(NBLK):
                tb = tbs[b]
                nA = splits[b]
                s = b % 3
                o = int(offs[b])
                if b + 1 < NBLK:
                    emit_gather(b + 1)

                mT_t = work.tile([128, TBMAX * 128], F8, tag="mT")
                nc.sync.dma_start(
                    out=mT_t[:, : tb * 128], in_=mT_d[:, o * 128 : (o + tb) * 128])
                mE_t = work.tile([128, TBMAX * 128], F8, tag="mE")
                nc.sync.dma_start(
                    out=mE_t[:, : tb * 128], in_=mE_d[:, o * 128 : (o + tb) * 128])

                # v_full = xl[src] + xr[dst] on PE: identity-copy + one-hot
                vsb = work.tile([128, TBMAX, DW], BF, tag="vsb")
                hh = (tb + 1) // 2
                for piece in range(2):
                    t0 = piece * hh
                    t1 = min(tb, t0 + hh)
                    if t0 >= t1:
                        continue
                    ps = psum_v.tile([128, hh, DW], F32, tag="psv")
                    for t in range(t0, t1):
                        nc.tensor.matmul(
                            out=ps[:, t - t0, :],
                            lhsT=mT_t[:, t * 128 : (t + 1) * 128],
                            rhs=xr_sb[:, b, :],
                            start=True, stop=False)
                        rhs_v = (vA[:, s, t, :] if t < nA
                                 else vB[:, s, t - nA, :])
                        nc.tensor.matmul(
                            out=ps[:, t - t0, :],
                            lhsT=ident8[:],
                            rhs=rhs_v,
                            start=False, stop=True)
                    nc.scalar.copy(out=vsb[:, t0:t1, :], in_=ps[:, : t1 - t0, :])

                # logits: att.lrelu(v) = 0.6*att.(v + (2/3)*|v|)
                #   a23 = Abs(v * 2/3) on ACT; u = v + a23 (DVE 2x);
                #   p = u * att_rep  (att_rep prescaled by 0.6)
                a23 = work.tile([128, TBMAX, DW], BF, tag="a23")
                nc.scalar.activation(
                    out=a23[:, :tb, :], in_=vsb[:, :tb, :], func=ACTF.Abs,
                    scale=(1.0 - NEG_SLOPE) / (1.0 + NEG_SLOPE))
                lall = work.tile([128, TBMAX, DW], BF, tag="lall")
                nc.vector.tensor_tensor(
                    out=lall[:, :tb, :], in0=vsb[:, :tb, :],
                    in1=a23[:, :tb, :], op=A.add)
                p_t = work.tile([128, TBMAX, DW], BF, tag="patt")
                nc.vector.tensor_tensor(
                    out=p_t[:, :tb, :], in0=lall[:, :tb, :],
                    in1=attrep_s[l][:, :tb, :], op=A.mult)
                lg = work.tile([128, TBMAX, H], F32, tag="lg")
                nc.vector.tensor_reduce(
                    out=lg[:, :tb, :],
                    in_=p_t[:, :tb, :].rearrange("p t (h c) -> p t h c", h=H),
                    axis=mybir.AxisListType.X, op=A.add)
                e_t = work.tile([128, TBMAX, H, 1], BF, tag="expv")
                nc.scalar.activation(out=e_t[:, :tb, :, :], in_=lg[:, :tb, :],
                                     func=ACTF.Exp)
                w_all = work.tile([128, TBMAX, W], BF, tag="wall")
                nc.scalar.copy(out=w_all[:, :tb, :H], in_=e_t[:, :tb, :, :])
                e_rep = work.tile([128, TBMAX, DW], BF, tag="erep")
                nc.scalar.activation(
                    out=e_rep[:, :tb, :].rearrange("p t (h c) -> p t h c", h=H),
                    in_=e_t[:, :tb, :, :].to_broadcast([128, tb, H, DW // H]),
                    func=ACTF.Copy)
                nc.vector.tensor_tensor(
                    out=w_all[:, :tb, H:], in0=vsb[:, :tb, :],
                    in1=e_rep[:, :tb, :], op=A.mult)

                o_ps = psum_g.tile([128, W], F32, tag="agg")
                for t in range(tb):
                    nc.tensor.matmul(
                        out=o_ps[:],
                        lhsT=mE_t[:, t * 128 : (t + 1) * 128],
                        rhs=w_all[:, t, :],
                        start=(t == 0), stop=(t == tb - 1))

                # node side (first D of the DW-padded payload)
                dn = nodep.tile([128, H], F32, tag="dn")
                nc.vector.tensor_scalar(
                    out=dn[:], in0=o_ps[:, :H], scalar1=1e-30, scalar2=None, op0=A.add)
                rc = nodep.tile([128, H], F32, tag="rc")
                nc.vector.reciprocal(out=rc[:], in_=dn[:])
                onorm = nodep.tile([128, H, C], F32, tag="onorm")
                nc.vector.tensor_tensor(
                    out=onorm[:],
                    in0=o_ps[:, H : H + D].rearrange("p (h c) -> p h c", h=H),
                    in1=rc[:].rearrange("p (h o) -> p h o", h=H).to_broadcast(
                        [128, H, C]),
                    op=A.mult)
                mk = nodep.tile([128, 1], F32, tag="mk")
                nc.vector.tensor_scalar(
                    out=mk[:], in0=o_ps[:, 0:1], scalar1=0.0, scalar2=-1.0,
                    op0=A.is_gt, op1=A.mult)
                hsub = nodep.tile([128, D], F32, tag="hsub")
                nc.vector.scalar_tensor_tensor(
                    out=hsub[:], in0=xr_sb[:, b, :D], scalar=mk[:, :1],
                    in1=onorm[:].rearrange("p h c -> p (h c)"),
                    op0=A.mult, op1=A.add)
                hb = nodep.tile([128, D], F32, tag="hb")
                nc.vector.tensor_tensor(
                    out=hb[:], in0=hsub[:], in1=biasout_s[l][:], op=A.add)
                if use_elu:
                    amax = nodep.tile([128, D], F32, tag="amax")
                    nc.scalar.activation(out=amax[:], in_=hb[:], func=ACTF.Relu)
                    amin = nodep.tile([128, D], F32, tag="amin")
                    nc.vector.tensor_scalar(
                        out=amin[:], in0=hb[:], scalar1=0.0, scalar2=None, op0=A.min)
                    aexp = nodep.tile([128, D], F32, tag="aexp")
                    nc.scalar.activation(out=aexp[:], in_=amin[:], func=ACTF.Exp)
                    nc.vector.scalar_tensor_tensor(
                        out=h_sb[:, b, :D], in0=amax[:], scalar=-1.0, in1=aexp[:],
                        op0=A.add, op1=A.add)
                else:
                    h2b = nodep.tile([128, D], BF, tag="h2b")
                    nc.vector.tensor_scalar(
                        out=h2b[:], in0=hb[:], scalar1=0.0, scalar2=None, op0=A.add)
                    pm_t = nodep.tile([128, G], BF, tag="pmt")
                    nc.sync.dma_start(
                        out=pm_t[:], in_=pool_mask[b * 128 : (b + 1) * 128, :])
                    nc.tensor.matmul(
                        out=pool_ps[:], lhsT=pm_t[:], rhs=h2b[:],
                        start=(b == 0), stop=(b == NBLK - 1))

                if l < 2:
                    phase_a(l + 1, b, ccw)
                    if b == NBLK - 1:
                        fire_cc(l + 1, ccw)

        pool_sb = nodep.tile([G, OUT_CH], F32, tag="poolsb")
        nc.scalar.copy(out=pool_sb[:], in_=pool_ps[:])
        nc.sync.dma_start(out=pool_out[:], in_=pool_sb[:])

    nc.compile()
    return nc


# ---------------------------------------------------------------- runner
_BUILD_CACHE = {}


def run(inp, trace=False):
    from concourse import bass_utils

    maps, counts, tbs, splits, kreals = prep(
        inp["x"], inp["edge_index"], inp["batch"])
    w = prep_weights(inp)
    for m in maps:
        m.update(w)

    key = (tbs, splits, kreals)
    if key not in _BUILD_CACHE:
        _BUILD_CACHE[key] = build(tbs, splits, kreals)
    nc = _BUILD_CACHE[key]

    res = bass_utils.run_bass_kernel_spmd(
        nc, maps, core_ids=list(range(NCORES)), trace=trace
    )
    total = np.zeros((N_GRAPHS, OUT_CH), np.float64)
    for k in range(NCORES):
        total += res.results[k]["pool_out"].astype(np.float64)
    out = (total / np.maximum(counts, 1.0)[:, None]).astype(np.float32)
    return out, res


def kernel(**inputs) -> np.ndarray:
    out, _ = run(inputs, trace=False)
    return out
